# revision 8
# baseline (speedup 1.0000x reference)
"""CoAttention ImageDNS kernel for Trainium2 (8 NeuronCores, Bass/Tile).

Math: the reference computes two additive-attention blocks. In both, the
softmax'd score is  score[b, q, k] = f(q-side)[b, q] + g(k-side)[b, k] + c,
and softmax over k is invariant to the q-dependent (and constant) terms, so
the attention weights are independent of the query index:

  visual_att[b, s, :]  = softmax_r( wB . tanh(W_i1 @ img[b, r]) )
  textual_att[b, i, :] = softmax_j( wD . tanh(W_d2 @ dns[b, j]) )

Hence both outputs are per-batch rank-1 broadcasts:

  att_img_features[b, s, :] = visual_att[b]  @ img[b]   (same for all s)
  att_dns_features[b, i, :] = textual_att[b] @ dns[b]   (same for all i)

W_d1/b_d1/w_att1[:H]/b_att1/W_i2/b_i2/w_att2[:H]/b_att2 cancel entirely.

Device computes only the per-batch H-vector for each side; the host
broadcasts it along S.  PE does only the irreducible work: the two HxH
projections, weight-stationary (stationary = W^T chunk, moving = x^T
pair-chunk: dns N=512 per batch, img N=392 per batch-pair) plus the w-dot
score matvec.  exp+sum run as one ACT op (accum_out); the attention
application is a DVE multiply-accumulate over the same transposed x tiles.
All DRAM operands are partition-major (each partition's bytes contiguous)
and input DMAs are spread over the sync/scalar (HWDGE) and gpsimd (SWDGE)
rings so transfers run in parallel and the first matmul starts early.
Score blocks are emitted directly after their pair's projection pass so the
DVE/ACT softmax+apply work overlaps the next projection pass.

Sharding: pure data-parallel over batch, 4 batches per core, no collectives.
"""

import sys
import numpy as np
import ml_dtypes

_BF16 = ml_dtypes.bfloat16

for _p in ("/opt/trn_rl_repo", "/root/.axon_site/_ro/trn_rl_repo"):
    if _p not in sys.path:
        sys.path.append(_p)

B, S, R, H = 32, 512, 196, 1024
NCORES = 8
BLOC = B // NCORES          # batches per core
NPAIR = BLOC // 2
HC = H // 128               # contraction chunks (h on partitions)
OC = H // 128               # output chunks (o on partitions after transpose)

_CACHE = {}


def build_nc():
    from concourse import bacc, mybir
    from concourse import tile

    f32, f16 = mybir.dt.float32, mybir.dt.bfloat16
    Act = mybir.ActivationFunctionType
    Alu = mybir.AluOpType

    nc = bacc.Bacc("TRN2", target_bir_lowering=False, debug=False)

    ROWS = {"dns": S, "img": R}
    PW = {"dns": 2 * S, "img": 2 * R}   # pair width per hc chunk

    # all DRAM layouts partition-major: [128, free...] contiguous per partition
    wt_dns = nc.dram_tensor("wt_dns", [128, OC * HC * 128], f16,
                            kind="ExternalInput")
    wt_img = nc.dram_tensor("wt_img", [128, OC * HC * 128], f16,
                            kind="ExternalInput")
    xp_dns = nc.dram_tensor("xp_dns", [NPAIR, 128, HC * 2 * S], f16,
                            kind="ExternalInput")
    xp_img = nc.dram_tensor("xp_img", [NPAIR, 128, HC * 2 * R], f16,
                            kind="ExternalInput")
    wc_dns = nc.dram_tensor("wc_dns", [128, OC], f16, kind="ExternalInput")
    wc_img = nc.dram_tensor("wc_img", [128, OC], f16, kind="ExternalInput")
    out_dns = nc.dram_tensor("out_dns", [BLOC, 128, OC], f32, kind="ExternalOutput")
    out_img = nc.dram_tensor("out_img", [BLOC, 128, OC], f32, kind="ExternalOutput")

    with tile.TileContext(nc) as tc:
        with (
            tc.tile_pool(name="const", bufs=1) as cpool,
            tc.tile_pool(name="xts", bufs=1) as xtpool,
            tc.tile_pool(name="ths", bufs=1) as thpool,
            tc.tile_pool(name="small", bufs=2) as spool,
            tc.tile_pool(name="pp", bufs=3, space="PSUM") as ppool,
            tc.tile_pool(name="ps", bufs=2, space="PSUM") as pscore,
        ):
            wt_sb, wc_sb = {}, {}
            xp_sb = {"dns": {}, "img": {}}
            th_sb = {"dns": {}, "img": {}}

            # -- PE warmup: dummy matmuls on memset scratch keep HAM at full
            # clock through the DMA-bound start (no data dependencies) --
            scratch = cpool.tile([128, 640], f16, name="warm_scratch")
            nc.vector.memset(scratch[:, :], 0.0)
            warm_ps = pscore.tile([128, 512], f32, name="warm_ps", tag="sc")
            for _ in range(32):
                nc.tensor.matmul(warm_ps[:, :], lhsT=scratch[:, 0:128],
                                 rhs=scratch[:, 128:640], start=True, stop=True)

            # -- SWDGE (gpsimd) ring: weights; critical oc=0 chunk first --
            def load_wt(side):
                dram = {"dns": wt_dns, "img": wt_img}[side]
                w = cpool.tile([128, OC * HC * 128], f16, name=f"wt_{side}_sb")
                nc.gpsimd.dma_start(out=w[:, 0:H], in_=dram[:, 0:H])
                nc.gpsimd.dma_start(out=w[:, H:], in_=dram[:, H:])
                wt_sb[side] = w

            def load_wc(side):
                dram = {"dns": wc_dns, "img": wc_img}[side]
                w = cpool.tile([128, OC], f16, name=f"wc_{side}_sb")
                nc.gpsimd.dma_start(out=w[:, :], in_=dram[:, :])
                wc_sb[side] = w

            # -- HWDGE rings: activations (sync=dns, scalar=img) --
            def load_xp(side, pair, splits, eng):
                pw = PW[side]
                dram = {"dns": xp_dns, "img": xp_img}[side]
                x = xtpool.tile([128, HC * pw], f16, name=f"xp_{side}_{pair}",
                                tag=f"xp_{side}_{pair}")
                for h0, h1 in splits:
                    eng.dma_start(out=x[:, h0 * pw:h1 * pw],
                                  in_=dram[pair, :, h0 * pw:h1 * pw])
                xp_sb[side][pair] = x

            # need-ordered loads (SDMA round-robins all queues with pending
            # work, so issue strictly in phase order: img p0 / dns / img p1).
            # Each projection pass touches its whole xp tile in the first
            # oc-group, so bulk transfers beat slab trickle.
            load_wt("img")
            load_xp("img", 0, [(0, 8)], nc.sync)
            load_wc("img")
            load_wt("dns")
            load_xp("dns", 0, [(0, 8)], nc.sync)
            load_xp("dns", 1, [(0, 8)], nc.sync)
            load_wc("dns")
            load_xp("img", 1, [(0, 8)], nc.sync)

            def proj_pair(side, pair):
                """projection for batch pair: psum [o, bi*rows + r], tanh."""
                n = ROWS[side]
                pw = PW[side]
                wt = wt_sb[side]
                xp = xp_sb[side][pair]
                th = thpool.tile([128, OC * pw], f16, name=f"th_{side}_{pair}",
                                 tag=f"th_{side}_{pair}")
                th_sb[side][pair] = th
                for oc in range(OC):
                    ps = ppool.tile([128, 1024], f32,
                                    name=f"proj_{side}_{pair}_{oc}", tag="pp")
                    for hc in range(HC):
                        lhsT = wt[:, oc * H + hc * 128: oc * H + (hc + 1) * 128]
                        if side == "dns":
                            for bi in range(2):
                                nc.tensor.matmul(
                                    ps[:, bi * 512: bi * 512 + n],
                                    lhsT=lhsT,
                                    rhs=xp[:, hc * pw + bi * n: hc * pw + (bi + 1) * n],
                                    start=(hc == 0), stop=(hc == HC - 1))
                        else:
                            nc.tensor.matmul(
                                ps[:, 0:pw], lhsT=lhsT,
                                rhs=xp[:, hc * pw:(hc + 1) * pw],
                                start=(hc == 0), stop=(hc == HC - 1))
                    if side == "dns":
                        nc.scalar.activation(th[:, oc * pw:(oc + 1) * pw],
                                             ps[:, 0:1024], Act.Tanh)
                    else:
                        nc.scalar.activation(th[:, oc * pw:(oc + 1) * pw],
                                             ps[:, 0:pw], Act.Tanh)

            def attend(side, b, sps, score_off):
                """exp+sum, broadcast, DVE apply, scale, output DMA."""
                n = ROWS[side]
                pw = PW[side]
                pair, bi = divmod(b, 2)
                xp = xp_sb[side][pair]
                out_d = {"dns": out_dns, "img": out_img}[side]

                e_row = spool.tile([1, 512], f16, name=f"e_{side}_{b}", tag="e")
                zsum = spool.tile([1, 1], f32, name=f"z_{side}_{b}", tag="z")
                nc.scalar.activation(e_row[0:1, 0:n],
                                     sps[0:1, score_off:score_off + n], Act.Exp,
                                     accum_out=zsum[0:1, 0:1])
                rec = spool.tile([1, 1], f32, name=f"rc_{side}_{b}", tag="rc")
                nc.vector.reciprocal(rec[0:1, 0:1], zsum[0:1, 0:1])
                rb = spool.tile([128, 1], f32, name=f"rb_{side}_{b}", tag="rb")
                nc.gpsimd.partition_broadcast(rb[:, 0:1], rec[0:1, 0:1])
                eb = spool.tile([128, 512], f16, name=f"eb_{side}_{b}", tag="eb")
                nc.gpsimd.partition_broadcast(eb[:, 0:n], e_row[0:1, 0:n])

                oraw = spool.tile([128, OC], f32, name=f"or_{side}_{b}", tag="oraw")
                scr = spool.tile([128, 512], f16, name=f"scr_{side}_{b}",
                                 tag=f"scr_{side}")
                for hc in range(HC):
                    nc.vector.scalar_tensor_tensor(
                        out=scr[:, 0:n],
                        in0=xp[:, hc * pw + bi * n: hc * pw + (bi + 1) * n],
                        scalar=1.0, in1=eb[:, 0:n],
                        op0=Alu.mult, op1=Alu.mult,
                        accum_out=oraw[:, hc:hc + 1])
                osb = spool.tile([128, OC], f32, name=f"ov_{side}_{b}", tag="ov")
                nc.scalar.activation(osb[:, :], oraw[:, :], Act.Copy,
                                     scale=rb[:, 0:1])
                nc.sync.dma_start(out=out_d[b], in_=osb[:, :])

            def score_dns(b):
                pair, bi = divmod(b, 2)
                th = th_sb["dns"][pair]
                pw = PW["dns"]
                sps = pscore.tile([1, 512], f32, name=f"s_dns_{b}", tag="sc")
                for oc in range(OC):
                    nc.tensor.matmul(
                        sps[0:1, 0:S], lhsT=wc_sb["dns"][:, oc:oc + 1],
                        rhs=th[:, oc * pw + bi * S: oc * pw + (bi + 1) * S],
                        start=(oc == 0), stop=(oc == OC - 1))
                attend("dns", b, sps, 0)

            def score_img_pair(pair):
                th = th_sb["img"][pair]
                pw = PW["img"]
                sps = pscore.tile([1, 512], f32, name=f"s_img_{pair}", tag="sc")
                for oc in range(OC):
                    nc.tensor.matmul(
                        sps[0:1, 0:pw], lhsT=wc_sb["img"][:, oc:oc + 1],
                        rhs=th[:, oc * pw:(oc + 1) * pw],
                        start=(oc == 0), stop=(oc == OC - 1))
                attend("img", 2 * pair, sps, 0)
                attend("img", 2 * pair + 1, sps, R)

            # -- schedule: img p0 first (smallest working set — matches the
            # DMA-bound start), dns passes in the middle, img p1 last so the
            # exposed tail is the short img chain. Each score block directly
            # follows its pair's pass and overlaps the next pass. --
            proj_pair("img", 0)
            score_img_pair(0)
            proj_pair("dns", 0)
            score_dns(0)
            score_dns(1)
            proj_pair("dns", 1)
            score_dns(2)
            score_dns(3)
            proj_pair("img", 1)
            score_img_pair(1)

    nc.compile()
    return nc


def _get_nc():
    if "nc" not in _CACHE:
        _CACHE["nc"] = build_nc()
    return _CACHE["nc"]


def make_in_maps(inputs):
    dns = np.ascontiguousarray(np.asarray(inputs["dns_feature"], dtype=np.float32))
    img = np.ascontiguousarray(np.asarray(inputs["img_features"], dtype=np.float32))
    W_i1 = np.asarray(inputs["W_i1"], dtype=np.float32)
    W_d2 = np.asarray(inputs["W_d2"], dtype=np.float32)
    wB = np.asarray(inputs["w_att1"], dtype=np.float32)[H:]
    wD = np.asarray(inputs["w_att2"], dtype=np.float32)[H:]

    # W^T[h, o] partition-major: [128(p=h%128), oc, hc, o] flattened
    def wt_flat(W):
        WT = np.ascontiguousarray(W.T).reshape(HC, 128, OC, 128)  # [hc,p,oc,o]
        return np.ascontiguousarray(
            WT.transpose(1, 2, 0, 3).reshape(128, OC * HC * 128).astype(_BF16))

    wt_dns = wt_flat(W_d2)
    wt_img = wt_flat(W_i1)
    wc_dns = np.ascontiguousarray(wD.reshape(OC, 128).T.astype(_BF16))
    wc_img = np.ascontiguousarray(wB.reshape(OC, 128).T.astype(_BF16))

    # x^T pair tiles, partition-major: [pair, 128, hc*(2n)] with [b_even|b_odd]
    def xp_tiles(x, n):
        xt = x.transpose(0, 2, 1).reshape(B, HC, 128, n)        # [b, hc, p, n]
        xt = xt.reshape(B // 2, 2, HC, 128, n)                  # [pair, bi, ...]
        return np.ascontiguousarray(
            xt.transpose(0, 3, 2, 1, 4).reshape(B // 2, 128, HC * 2 * n)
            .astype(_BF16))

    xp_dns = xp_tiles(dns, S)
    xp_img = xp_tiles(img, R)

    in_maps = []
    for k in range(NCORES):
        sl = slice(k * NPAIR, (k + 1) * NPAIR)
        in_maps.append({
            "wt_dns": wt_dns,
            "wt_img": wt_img,
            "xp_dns": np.ascontiguousarray(xp_dns[sl]),
            "xp_img": np.ascontiguousarray(xp_img[sl]),
            "wc_dns": wc_dns,
            "wc_img": wc_img,
        })
    return in_maps


def kernel(**inputs):
    from concourse.bass_utils import run_bass_kernel_spmd

    nc = _get_nc()
    in_maps = make_in_maps(inputs)
    res = run_bass_kernel_spmd(nc, in_maps, list(range(NCORES))).results
    # device emits [128, OC] per (batch, side): vec[h] = arr[h % 128, h // 128]
    dns_v = np.concatenate(
        [res[k]["out_dns"].transpose(0, 2, 1).reshape(BLOC, H)
         for k in range(NCORES)], axis=0)
    img_v = np.concatenate(
        [res[k]["out_img"].transpose(0, 2, 1).reshape(BLOC, H)
         for k in range(NCORES)], axis=0)
    att_dns = np.broadcast_to(dns_v[:, None, :], (B, S, H))
    att_img = np.broadcast_to(img_v[:, None, :], (B, S, H))
    return att_dns, att_img


# revision 9
# speedup vs baseline: 1.1655x; 1.1655x over previous
"""CoAttention ImageDNS kernel for Trainium2 (8 NeuronCores, Bass/Tile).

Math: the reference computes two additive-attention blocks. In both, the
softmax'd score is  score[b, q, k] = f(q-side)[b, q] + g(k-side)[b, k] + c,
and softmax over k is invariant to the q-dependent (and constant) terms, so
the attention weights are independent of the query index:

  visual_att[b, s, :]  = softmax_r( wB . tanh(W_i1 @ img[b, r]) )
  textual_att[b, i, :] = softmax_j( wD . tanh(W_d2 @ dns[b, j]) )

Hence both outputs are per-batch rank-1 broadcasts:

  att_img_features[b, s, :] = visual_att[b]  @ img[b]   (same for all s)
  att_dns_features[b, i, :] = textual_att[b] @ dns[b]   (same for all i)

W_d1/b_d1/w_att1[:H]/b_att1/W_i2/b_i2/w_att2[:H]/b_att2 cancel entirely.

Device computes only the per-batch H-vector for each side; the host
broadcasts it along S.  PE does only the irreducible work: the two HxH
projections, weight-stationary (stationary = W^T chunk, moving = x^T
pair-chunk: dns N=512 per batch, img N=392 per batch-pair) plus the w-dot
score matvec.  exp+sum run as one ACT op (accum_out); the attention
application is a DVE multiply-accumulate over the same transposed x tiles.
All DRAM operands are partition-major (each partition's bytes contiguous)
and input DMAs are spread over the sync/scalar (HWDGE) and gpsimd (SWDGE)
rings so transfers run in parallel and the first matmul starts early.
Score blocks are emitted directly after their pair's projection pass so the
DVE/ACT softmax+apply work overlaps the next projection pass.

Sharding: pure data-parallel over batch, 4 batches per core, no collectives.
"""

import sys
import numpy as np
import ml_dtypes

_BF16 = ml_dtypes.bfloat16

for _p in ("/opt/trn_rl_repo", "/root/.axon_site/_ro/trn_rl_repo"):
    if _p not in sys.path:
        sys.path.append(_p)

B, S, R, H = 32, 512, 196, 1024
NCORES = 8
BLOC = B // NCORES          # batches per core
NPAIR = BLOC // 2
HC = H // 128               # contraction chunks (h on partitions)
OC = H // 128               # output chunks (o on partitions after transpose)

_CACHE = {}


def build_nc():
    from concourse import bacc, mybir
    from concourse import tile

    f32, f16 = mybir.dt.float32, mybir.dt.bfloat16
    Act = mybir.ActivationFunctionType
    Alu = mybir.AluOpType

    nc = bacc.Bacc("TRN2", target_bir_lowering=False, debug=False)

    ROWS = {"dns": S, "img": R}
    PW = {"dns": 2 * S, "img": 2 * R}   # pair width per hc chunk

    # all DRAM layouts partition-major: [128, free...] contiguous per partition
    wt_dns = nc.dram_tensor("wt_dns", [128, OC * HC * 128], f16,
                            kind="ExternalInput")
    wt_img = nc.dram_tensor("wt_img", [128, OC * HC * 128], f16,
                            kind="ExternalInput")
    xp_dns = nc.dram_tensor("xp_dns", [NPAIR, 128, HC * 2 * S], f16,
                            kind="ExternalInput")
    xp_img = nc.dram_tensor("xp_img", [NPAIR, 128, HC * 2 * R], f16,
                            kind="ExternalInput")
    wc_dns = nc.dram_tensor("wc_dns", [128, OC], f16, kind="ExternalInput")
    wc_img = nc.dram_tensor("wc_img", [128, OC], f16, kind="ExternalInput")
    out_dns = nc.dram_tensor("out_dns", [BLOC, 128, OC], f32, kind="ExternalOutput")
    out_img = nc.dram_tensor("out_img", [BLOC, 128, OC], f32, kind="ExternalOutput")

    with tile.TileContext(nc) as tc:
        with (
            tc.tile_pool(name="const", bufs=1) as cpool,
            tc.tile_pool(name="xts", bufs=1) as xtpool,
            tc.tile_pool(name="ths", bufs=1) as thpool,
            tc.tile_pool(name="small", bufs=2) as spool,
            tc.tile_pool(name="pp", bufs=3, space="PSUM") as ppool,
            tc.tile_pool(name="ps", bufs=2, space="PSUM") as pscore,
        ):
            wt_sb, wc_sb = {}, {}
            xp_sb = {"dns": {}, "img": {}}
            th_sb = {"dns": {}, "img": {}}

            # -- PE warmup: dummy matmuls on memset scratch keep HAM at full
            # clock through the DMA-bound start (no data dependencies) --
            scratch = cpool.tile([128, 640], f16, name="warm_scratch")
            nc.vector.memset(scratch[:, :], 0.0)
            warm_ps = pscore.tile([128, 512], f32, name="warm_ps", tag="sc")
            for _ in range(12):
                nc.tensor.matmul(warm_ps[:, :], lhsT=scratch[:, 0:128],
                                 rhs=scratch[:, 128:640], start=True, stop=True)

            # -- single HWDGE (sync) ring for all loads, strictly in need
            # order (FIFO transfer order; SWDGE measured far slower).
            # Weights split per-oc so completion sems fire just ahead of
            # each oc-group's matmuls. --
            def alloc_wt(side):
                w = cpool.tile([128, OC * HC * 128], f16, name=f"wt_{side}_sb")
                wt_sb[side] = w
                return w

            def load_wt_oc(side, ocs):
                dram = {"dns": wt_dns, "img": wt_img}[side]
                for oc in ocs:
                    nc.sync.dma_start(out=wt_sb[side][:, oc * H:(oc + 1) * H],
                                      in_=dram[:, oc * H:(oc + 1) * H])

            def load_wc(side):
                dram = {"dns": wc_dns, "img": wc_img}[side]
                w = cpool.tile([128, OC], f16, name=f"wc_{side}_sb")
                nc.sync.dma_start(out=w[:, :], in_=dram[:, :])
                wc_sb[side] = w

            def load_xp(side, pair):
                pw = PW[side]
                dram = {"dns": xp_dns, "img": xp_img}[side]
                x = xtpool.tile([128, HC * pw], f16, name=f"xp_{side}_{pair}",
                                tag=f"xp_{side}_{pair}")
                nc.sync.dma_start(out=x[:, :], in_=dram[pair, :, :])
                xp_sb[side][pair] = x

            alloc_wt("img")
            alloc_wt("dns")
            load_wt_oc("img", [0])
            load_xp("img", 0)
            load_wt_oc("img", range(1, OC))
            load_wc("img")
            load_xp("dns", 0)
            load_wt_oc("dns", range(OC))
            load_wc("dns")
            load_xp("dns", 1)
            load_xp("img", 1)

            def proj_pair(side, pair):
                """projection for batch pair: psum [o, bi*rows + r], tanh."""
                n = ROWS[side]
                pw = PW[side]
                wt = wt_sb[side]
                xp = xp_sb[side][pair]
                th = thpool.tile([128, OC * pw], f16, name=f"th_{side}_{pair}",
                                 tag=f"th_{side}_{pair}")
                th_sb[side][pair] = th
                for oc in range(OC):
                    ps = ppool.tile([128, 1024], f32,
                                    name=f"proj_{side}_{pair}_{oc}", tag="pp")
                    for hc in range(HC):
                        lhsT = wt[:, oc * H + hc * 128: oc * H + (hc + 1) * 128]
                        if side == "dns":
                            for bi in range(2):
                                nc.tensor.matmul(
                                    ps[:, bi * 512: bi * 512 + n],
                                    lhsT=lhsT,
                                    rhs=xp[:, hc * pw + bi * n: hc * pw + (bi + 1) * n],
                                    start=(hc == 0), stop=(hc == HC - 1))
                        else:
                            nc.tensor.matmul(
                                ps[:, 0:pw], lhsT=lhsT,
                                rhs=xp[:, hc * pw:(hc + 1) * pw],
                                start=(hc == 0), stop=(hc == HC - 1))
                    if side == "dns":
                        nc.scalar.activation(th[:, oc * pw:(oc + 1) * pw],
                                             ps[:, 0:1024], Act.Tanh)
                    else:
                        nc.scalar.activation(th[:, oc * pw:(oc + 1) * pw],
                                             ps[:, 0:pw], Act.Tanh)

            def attend(side, b, sps, score_off):
                """exp+sum, broadcast, DVE apply, scale, output DMA."""
                n = ROWS[side]
                pw = PW[side]
                pair, bi = divmod(b, 2)
                xp = xp_sb[side][pair]
                out_d = {"dns": out_dns, "img": out_img}[side]

                e_row = spool.tile([1, 512], f16, name=f"e_{side}_{b}", tag="e")
                zsum = spool.tile([1, 1], f32, name=f"z_{side}_{b}", tag="z")
                nc.scalar.activation(e_row[0:1, 0:n],
                                     sps[0:1, score_off:score_off + n], Act.Exp,
                                     accum_out=zsum[0:1, 0:1])
                rec = spool.tile([1, 1], f32, name=f"rc_{side}_{b}", tag="rc")
                nc.vector.reciprocal(rec[0:1, 0:1], zsum[0:1, 0:1])
                rb = spool.tile([128, 1], f32, name=f"rb_{side}_{b}", tag="rb")
                nc.gpsimd.partition_broadcast(rb[:, 0:1], rec[0:1, 0:1])
                eb = spool.tile([128, 512], f16, name=f"eb_{side}_{b}", tag="eb")
                nc.gpsimd.partition_broadcast(eb[:, 0:n], e_row[0:1, 0:n])

                oraw = spool.tile([128, OC], f32, name=f"or_{side}_{b}", tag="oraw")
                scr = spool.tile([128, 512], f16, name=f"scr_{side}_{b}",
                                 tag=f"scr_{side}")
                for hc in range(HC):
                    nc.vector.scalar_tensor_tensor(
                        out=scr[:, 0:n],
                        in0=xp[:, hc * pw + bi * n: hc * pw + (bi + 1) * n],
                        scalar=1.0, in1=eb[:, 0:n],
                        op0=Alu.mult, op1=Alu.mult,
                        accum_out=oraw[:, hc:hc + 1])
                osb = spool.tile([128, OC], f32, name=f"ov_{side}_{b}", tag="ov")
                nc.scalar.activation(osb[:, :], oraw[:, :], Act.Copy,
                                     scale=rb[:, 0:1])
                nc.sync.dma_start(out=out_d[b], in_=osb[:, :])

            def score_dns(b):
                pair, bi = divmod(b, 2)
                th = th_sb["dns"][pair]
                pw = PW["dns"]
                sps = pscore.tile([1, 512], f32, name=f"s_dns_{b}", tag="sc")
                for oc in range(OC):
                    nc.tensor.matmul(
                        sps[0:1, 0:S], lhsT=wc_sb["dns"][:, oc:oc + 1],
                        rhs=th[:, oc * pw + bi * S: oc * pw + (bi + 1) * S],
                        start=(oc == 0), stop=(oc == OC - 1))
                attend("dns", b, sps, 0)

            def score_img_pair(pair):
                th = th_sb["img"][pair]
                pw = PW["img"]
                sps = pscore.tile([1, 512], f32, name=f"s_img_{pair}", tag="sc")
                for oc in range(OC):
                    nc.tensor.matmul(
                        sps[0:1, 0:pw], lhsT=wc_sb["img"][:, oc:oc + 1],
                        rhs=th[:, oc * pw:(oc + 1) * pw],
                        start=(oc == 0), stop=(oc == OC - 1))
                attend("img", 2 * pair, sps, 0)
                attend("img", 2 * pair + 1, sps, R)

            # -- schedule: img p0 first (smallest working set — matches the
            # DMA-bound start), dns passes in the middle, img p1 last so the
            # exposed tail is the short img chain. Each score block directly
            # follows its pair's pass and overlaps the next pass. --
            proj_pair("img", 0)
            score_img_pair(0)
            proj_pair("dns", 0)
            score_dns(0)
            score_dns(1)
            proj_pair("dns", 1)
            score_dns(2)
            score_dns(3)
            proj_pair("img", 1)
            score_img_pair(1)

    nc.compile()
    return nc


def _get_nc():
    if "nc" not in _CACHE:
        _CACHE["nc"] = build_nc()
    return _CACHE["nc"]


def make_in_maps(inputs):
    dns = np.ascontiguousarray(np.asarray(inputs["dns_feature"], dtype=np.float32))
    img = np.ascontiguousarray(np.asarray(inputs["img_features"], dtype=np.float32))
    W_i1 = np.asarray(inputs["W_i1"], dtype=np.float32)
    W_d2 = np.asarray(inputs["W_d2"], dtype=np.float32)
    wB = np.asarray(inputs["w_att1"], dtype=np.float32)[H:]
    wD = np.asarray(inputs["w_att2"], dtype=np.float32)[H:]

    # W^T[h, o] partition-major: [128(p=h%128), oc, hc, o] flattened
    def wt_flat(W):
        WT = np.ascontiguousarray(W.T).reshape(HC, 128, OC, 128)  # [hc,p,oc,o]
        return np.ascontiguousarray(
            WT.transpose(1, 2, 0, 3).reshape(128, OC * HC * 128).astype(_BF16))

    wt_dns = wt_flat(W_d2)
    wt_img = wt_flat(W_i1)
    wc_dns = np.ascontiguousarray(wD.reshape(OC, 128).T.astype(_BF16))
    wc_img = np.ascontiguousarray(wB.reshape(OC, 128).T.astype(_BF16))

    # x^T pair tiles, partition-major: [pair, 128, hc*(2n)] with [b_even|b_odd]
    def xp_tiles(x, n):
        xt = x.transpose(0, 2, 1).reshape(B, HC, 128, n)        # [b, hc, p, n]
        xt = xt.reshape(B // 2, 2, HC, 128, n)                  # [pair, bi, ...]
        return np.ascontiguousarray(
            xt.transpose(0, 3, 2, 1, 4).reshape(B // 2, 128, HC * 2 * n)
            .astype(_BF16))

    xp_dns = xp_tiles(dns, S)
    xp_img = xp_tiles(img, R)

    in_maps = []
    for k in range(NCORES):
        sl = slice(k * NPAIR, (k + 1) * NPAIR)
        in_maps.append({
            "wt_dns": wt_dns,
            "wt_img": wt_img,
            "xp_dns": np.ascontiguousarray(xp_dns[sl]),
            "xp_img": np.ascontiguousarray(xp_img[sl]),
            "wc_dns": wc_dns,
            "wc_img": wc_img,
        })
    return in_maps


def kernel(**inputs):
    from concourse.bass_utils import run_bass_kernel_spmd

    nc = _get_nc()
    in_maps = make_in_maps(inputs)
    res = run_bass_kernel_spmd(nc, in_maps, list(range(NCORES))).results
    # device emits [128, OC] per (batch, side): vec[h] = arr[h % 128, h // 128]
    dns_v = np.concatenate(
        [res[k]["out_dns"].transpose(0, 2, 1).reshape(BLOC, H)
         for k in range(NCORES)], axis=0)
    img_v = np.concatenate(
        [res[k]["out_img"].transpose(0, 2, 1).reshape(BLOC, H)
         for k in range(NCORES)], axis=0)
    att_dns = np.broadcast_to(dns_v[:, None, :], (B, S, H))
    att_img = np.broadcast_to(img_v[:, None, :], (B, S, H))
    return att_dns, att_img


# revision 14
# speedup vs baseline: 1.1662x; 1.0006x over previous
"""CoAttention ImageDNS kernel for Trainium2 (8 NeuronCores, Bass/Tile).

Math: the reference computes two additive-attention blocks. In both, the
softmax'd score is  score[b, q, k] = f(q-side)[b, q] + g(k-side)[b, k] + c,
and softmax over k is invariant to the q-dependent (and constant) terms, so
the attention weights are independent of the query index:

  visual_att[b, s, :]  = softmax_r( wB . tanh(W_i1 @ img[b, r]) )
  textual_att[b, i, :] = softmax_j( wD . tanh(W_d2 @ dns[b, j]) )

Hence both outputs are per-batch rank-1 broadcasts:

  att_img_features[b, s, :] = visual_att[b]  @ img[b]   (same for all s)
  att_dns_features[b, i, :] = textual_att[b] @ dns[b]   (same for all i)

W_d1/b_d1/w_att1[:H]/b_att1/W_i2/b_i2/w_att2[:H]/b_att2 cancel entirely.

Device computes only the per-batch H-vector for each side; the host
broadcasts it along S.  PE does only the irreducible work: the two HxH
projections, weight-stationary (stationary = W^T chunk, moving = x^T
pair-chunk: dns N=512 per batch, img N=392 per batch-pair) plus the w-dot
score matvec.  exp+sum run as one ACT op (accum_out); the attention
application is a DVE multiply-accumulate over the same transposed x tiles.
All DRAM operands are partition-major (each partition's bytes contiguous)
and input DMAs are spread over the sync/scalar (HWDGE) and gpsimd (SWDGE)
rings so transfers run in parallel and the first matmul starts early.
Score blocks are emitted directly after their pair's projection pass so the
DVE/ACT softmax+apply work overlaps the next projection pass.

Sharding: pure data-parallel over batch, 4 batches per core, no collectives.
"""

import sys
import numpy as np
import ml_dtypes

_BF16 = ml_dtypes.bfloat16

for _p in ("/opt/trn_rl_repo", "/root/.axon_site/_ro/trn_rl_repo"):
    if _p not in sys.path:
        sys.path.append(_p)

B, S, R, H = 32, 512, 196, 1024
NCORES = 8
BLOC = B // NCORES          # batches per core
NPAIR = BLOC // 2
HC = H // 128               # contraction chunks (h on partitions)
OC = H // 128               # output chunks (o on partitions after transpose)

_CACHE = {}


def build_nc():
    from concourse import bacc, mybir
    from concourse import tile

    f32, f16 = mybir.dt.float32, mybir.dt.bfloat16
    Act = mybir.ActivationFunctionType
    Alu = mybir.AluOpType

    nc = bacc.Bacc("TRN2", target_bir_lowering=False, debug=False)

    ROWS = {"dns": S, "img": R}
    PW = {"dns": 2 * S, "img": 2 * R}   # pair width per hc chunk

    # all DRAM layouts partition-major: [128, free...] contiguous per partition
    wt_dns = nc.dram_tensor("wt_dns", [128, OC * HC * 128], f16,
                            kind="ExternalInput")
    wt_img = nc.dram_tensor("wt_img", [128, OC * HC * 128], f16,
                            kind="ExternalInput")
    xp_dns = nc.dram_tensor("xp_dns", [NPAIR, 128, HC * 2 * S], f16,
                            kind="ExternalInput")
    xp_img = nc.dram_tensor("xp_img", [NPAIR, 128, HC * 2 * R], f16,
                            kind="ExternalInput")
    wc_dns = nc.dram_tensor("wc_dns", [128, OC], f16, kind="ExternalInput")
    wc_img = nc.dram_tensor("wc_img", [128, OC], f16, kind="ExternalInput")
    out_dns = nc.dram_tensor("out_dns", [BLOC, 128, OC], f32, kind="ExternalOutput")
    out_img = nc.dram_tensor("out_img", [BLOC, 128, OC], f32, kind="ExternalOutput")

    with tile.TileContext(nc) as tc:
        with (
            tc.tile_pool(name="const", bufs=1) as cpool,
            tc.tile_pool(name="xts", bufs=1) as xtpool,
            tc.tile_pool(name="ths", bufs=1) as thpool,
            tc.tile_pool(name="small", bufs=2) as spool,
            tc.tile_pool(name="pp", bufs=3, space="PSUM") as ppool,
            tc.tile_pool(name="ps", bufs=2, space="PSUM") as pscore,
        ):
            wt_sb, wc_sb = {}, {}
            xp_sb = {"dns": {}, "img": {}}
            th_sb = {"dns": {}, "img": {}}

            # -- PE warmup: dummy matmuls on memset scratch keep HAM at full
            # clock through the DMA-bound start (no data dependencies) --
            scratch = cpool.tile([128, 640], f16, name="warm_scratch")
            nc.gpsimd.memset(scratch[:, :], 0.0)
            warm_ps = pscore.tile([128, 512], f32, name="warm_ps", tag="sc")
            for _ in range(16):
                nc.tensor.matmul(warm_ps[:, :], lhsT=scratch[:, 0:128],
                                 rhs=scratch[:, 128:640], start=True, stop=True)

            # -- single HWDGE (sync) ring for all loads, strictly in need
            # order (FIFO transfer order; SWDGE measured far slower).
            # Weights split per-oc so completion sems fire just ahead of
            # each oc-group's matmuls. --
            def alloc_wt(side):
                w = cpool.tile([128, OC * HC * 128], f16, name=f"wt_{side}_sb")
                wt_sb[side] = w
                return w

            def load_wt_oc(side, ocs):
                dram = {"dns": wt_dns, "img": wt_img}[side]
                for oc in ocs:
                    nc.sync.dma_start(out=wt_sb[side][:, oc * H:(oc + 1) * H],
                                      in_=dram[:, oc * H:(oc + 1) * H])

            def load_wc(side):
                dram = {"dns": wc_dns, "img": wc_img}[side]
                w = cpool.tile([128, OC], f16, name=f"wc_{side}_sb")
                nc.sync.dma_start(out=w[:, :], in_=dram[:, :])
                wc_sb[side] = w

            def load_xp(side, pair):
                pw = PW[side]
                dram = {"dns": xp_dns, "img": xp_img}[side]
                x = xtpool.tile([128, HC * pw], f16, name=f"xp_{side}_{pair}",
                                tag=f"xp_{side}_{pair}")
                nc.sync.dma_start(out=x[:, :], in_=dram[pair, :, :])
                xp_sb[side][pair] = x

            alloc_wt("img")
            alloc_wt("dns")
            load_wt_oc("img", [0])
            load_xp("img", 0)
            load_wt_oc("img", range(1, OC))
            load_wc("img")
            load_xp("dns", 0)
            load_wt_oc("dns", range(OC))
            load_wc("dns")
            load_xp("dns", 1)
            load_xp("img", 1)

            def proj_pair(side, pair):
                """projection for batch pair: psum [o, bi*rows + r], tanh."""
                n = ROWS[side]
                pw = PW[side]
                wt = wt_sb[side]
                xp = xp_sb[side][pair]
                th = thpool.tile([128, OC * pw], f16, name=f"th_{side}_{pair}",
                                 tag=f"th_{side}_{pair}")
                th_sb[side][pair] = th
                for oc in range(OC):
                    ps = ppool.tile([128, 1024], f32,
                                    name=f"proj_{side}_{pair}_{oc}", tag="pp")
                    for hc in range(HC):
                        lhsT = wt[:, oc * H + hc * 128: oc * H + (hc + 1) * 128]
                        if side == "dns":
                            for bi in range(2):
                                nc.tensor.matmul(
                                    ps[:, bi * 512: bi * 512 + n],
                                    lhsT=lhsT,
                                    rhs=xp[:, hc * pw + bi * n: hc * pw + (bi + 1) * n],
                                    start=(hc == 0), stop=(hc == HC - 1))
                        else:
                            nc.tensor.matmul(
                                ps[:, 0:pw], lhsT=lhsT,
                                rhs=xp[:, hc * pw:(hc + 1) * pw],
                                start=(hc == 0), stop=(hc == HC - 1))
                    if side == "dns":
                        nc.scalar.activation(th[:, oc * pw:(oc + 1) * pw],
                                             ps[:, 0:1024], Act.Tanh)
                    else:
                        nc.scalar.activation(th[:, oc * pw:(oc + 1) * pw],
                                             ps[:, 0:pw], Act.Tanh)

            def attend(side, b, sps, score_off, eng=None):
                """exp+sum, broadcast, apply (eng: DVE default), scale, DMA."""
                eng = eng or nc.vector
                n = ROWS[side]
                pw = PW[side]
                pair, bi = divmod(b, 2)
                xp = xp_sb[side][pair]
                out_d = {"dns": out_dns, "img": out_img}[side]

                e_row = spool.tile([1, 512], f16, name=f"e_{side}_{b}", tag="e")
                zsum = spool.tile([1, 1], f32, name=f"z_{side}_{b}", tag="z")
                nc.scalar.activation(e_row[0:1, 0:n],
                                     sps[0:1, score_off:score_off + n], Act.Exp,
                                     accum_out=zsum[0:1, 0:1])
                rec = spool.tile([1, 1], f32, name=f"rc_{side}_{b}", tag="rc")
                nc.vector.reciprocal(rec[0:1, 0:1], zsum[0:1, 0:1])
                rb = spool.tile([128, 1], f32, name=f"rb_{side}_{b}", tag="rb")
                nc.gpsimd.partition_broadcast(rb[:, 0:1], rec[0:1, 0:1])
                eb = spool.tile([128, 512], f16, name=f"eb_{side}_{b}", tag="eb")
                nc.gpsimd.partition_broadcast(eb[:, 0:n], e_row[0:1, 0:n])

                oraw = spool.tile([128, OC], f32, name=f"or_{side}_{b}", tag="oraw")
                scr = spool.tile([128, 512], f16, name=f"scr_{side}_{b}",
                                 tag=f"scr_{side}")
                for hc in range(HC):
                    eng.scalar_tensor_tensor(
                        out=scr[:, 0:n],
                        in0=xp[:, hc * pw + bi * n: hc * pw + (bi + 1) * n],
                        scalar=1.0, in1=eb[:, 0:n],
                        op0=Alu.mult, op1=Alu.mult,
                        accum_out=oraw[:, hc:hc + 1])
                osb = spool.tile([128, OC], f32, name=f"ov_{side}_{b}", tag="ov")
                nc.scalar.activation(osb[:, :], oraw[:, :], Act.Copy,
                                     scale=rb[:, 0:1])
                nc.sync.dma_start(out=out_d[b], in_=osb[:, :])

            def score_dns(b):
                pair, bi = divmod(b, 2)
                th = th_sb["dns"][pair]
                pw = PW["dns"]
                sps = pscore.tile([1, 512], f32, name=f"s_dns_{b}", tag="sc")
                for oc in range(OC):
                    nc.tensor.matmul(
                        sps[0:1, 0:S], lhsT=wc_sb["dns"][:, oc:oc + 1],
                        rhs=th[:, oc * pw + bi * S: oc * pw + (bi + 1) * S],
                        start=(oc == 0), stop=(oc == OC - 1))
                attend("dns", b, sps, 0)

            def score_img_pair(pair):
                th = th_sb["img"][pair]
                pw = PW["img"]
                sps = pscore.tile([1, 512], f32, name=f"s_img_{pair}", tag="sc")
                for oc in range(OC):
                    nc.tensor.matmul(
                        sps[0:1, 0:pw], lhsT=wc_sb["img"][:, oc:oc + 1],
                        rhs=th[:, oc * pw:(oc + 1) * pw],
                        start=(oc == 0), stop=(oc == OC - 1))
                attend("img", 2 * pair, sps, 0)
                attend("img", 2 * pair + 1, sps, R)

            # -- schedule: img p0 first (smallest working set — matches the
            # DMA-bound start), dns passes in the middle, img p1 last so the
            # exposed tail is the short img chain. Each score block directly
            # follows its pair's pass and overlaps the next pass. --
            proj_pair("img", 0)
            score_img_pair(0)
            proj_pair("dns", 0)
            score_dns(0)
            score_dns(1)
            proj_pair("dns", 1)
            score_dns(2)
            score_dns(3)
            proj_pair("img", 1)
            score_img_pair(1)

    nc.compile()
    return nc


def _get_nc():
    if "nc" not in _CACHE:
        _CACHE["nc"] = build_nc()
    return _CACHE["nc"]


def make_in_maps(inputs):
    dns = np.ascontiguousarray(np.asarray(inputs["dns_feature"], dtype=np.float32))
    img = np.ascontiguousarray(np.asarray(inputs["img_features"], dtype=np.float32))
    W_i1 = np.asarray(inputs["W_i1"], dtype=np.float32)
    W_d2 = np.asarray(inputs["W_d2"], dtype=np.float32)
    wB = np.asarray(inputs["w_att1"], dtype=np.float32)[H:]
    wD = np.asarray(inputs["w_att2"], dtype=np.float32)[H:]

    # W^T[h, o] partition-major: [128(p=h%128), oc, hc, o] flattened
    def wt_flat(W):
        WT = np.ascontiguousarray(W.T).reshape(HC, 128, OC, 128)  # [hc,p,oc,o]
        return np.ascontiguousarray(
            WT.transpose(1, 2, 0, 3).reshape(128, OC * HC * 128).astype(_BF16))

    wt_dns = wt_flat(W_d2)
    wt_img = wt_flat(W_i1)
    wc_dns = np.ascontiguousarray(wD.reshape(OC, 128).T.astype(_BF16))
    wc_img = np.ascontiguousarray(wB.reshape(OC, 128).T.astype(_BF16))

    # x^T pair tiles, partition-major: [pair, 128, hc*(2n)] with [b_even|b_odd]
    def xp_tiles(x, n):
        xt = x.transpose(0, 2, 1).reshape(B, HC, 128, n)        # [b, hc, p, n]
        xt = xt.reshape(B // 2, 2, HC, 128, n)                  # [pair, bi, ...]
        return np.ascontiguousarray(
            xt.transpose(0, 3, 2, 1, 4).reshape(B // 2, 128, HC * 2 * n)
            .astype(_BF16))

    xp_dns = xp_tiles(dns, S)
    xp_img = xp_tiles(img, R)

    in_maps = []
    for k in range(NCORES):
        sl = slice(k * NPAIR, (k + 1) * NPAIR)
        in_maps.append({
            "wt_dns": wt_dns,
            "wt_img": wt_img,
            "xp_dns": np.ascontiguousarray(xp_dns[sl]),
            "xp_img": np.ascontiguousarray(xp_img[sl]),
            "wc_dns": wc_dns,
            "wc_img": wc_img,
        })
    return in_maps


def kernel(**inputs):
    from concourse.bass_utils import run_bass_kernel_spmd

    nc = _get_nc()
    in_maps = make_in_maps(inputs)
    res = run_bass_kernel_spmd(nc, in_maps, list(range(NCORES))).results
    # device emits [128, OC] per (batch, side): vec[h] = arr[h % 128, h // 128]
    dns_v = np.concatenate(
        [res[k]["out_dns"].transpose(0, 2, 1).reshape(BLOC, H)
         for k in range(NCORES)], axis=0)
    img_v = np.concatenate(
        [res[k]["out_img"].transpose(0, 2, 1).reshape(BLOC, H)
         for k in range(NCORES)], axis=0)
    att_dns = np.broadcast_to(dns_v[:, None, :], (B, S, H))
    att_img = np.broadcast_to(img_v[:, None, :], (B, S, H))
    return att_dns, att_img
